# revision 4
# baseline (speedup 1.0000x reference)
"""Trainium2 Bass kernel for nn_MemoryBank (top-k masked attention over a
64-slot memory bank), data-parallel over 8 NeuronCores.

Math (per token t, memory slot n):
    scores[t, n] = (q[t] @ W_q.T) . (mem[n] @ W_k.T) / sqrt(D) + bias[n]
    bias[n]      = max(log(importance[n] * 0.99**age[n]), -10)
    top-8 mask over n, softmax, retrieved = attn @ mem

Device-side formulation:
    P = W_q.T @ (mem @ W_k.T).T / sqrt(D)    (256x64, host-precomputed in f64)
    scores = q @ P + bias                     (PE: 2 fp32 matmuls + K=1 bias matmul)
    em  = exp(scores)                         (ACT, direct from PSUM)
    e8  = top-8 of em per row                 (DVE max8; exp is monotonic)
    em_m = (em >= e8[:,7]) * em ; Z = sum     (one fused DVE scalar_tensor_tensor)
    attn = em_m * (1/Z)                       (DVE)
    retrieved = attn @ mem                    (PE transpose + matmul)

Sharding: batch-parallel. Core c gets tokens [c*16384, (c+1)*16384) of the
flattened (B*S=131072) token axis. q is host-transposed to [D, tokens] so the
contraction dim lands on SBUF partitions with dense DMA.
"""

import sys

for _p in ("/opt/trn_rl_repo",):
    if _p not in sys.path:
        sys.path.insert(0, _p)

import numpy as np

import concourse.bass as bass
import concourse.mybir as mybir
import concourse.tile as tile
from concourse.masks import make_identity
from concourse.vector_clock import ScopedClock

# ---------------------------------------------------------------------------
# Problem constants (hardcoded per the spec; kernel.py must be self-contained)
B, S, D, N = 32, 4096, 256, 64
TOPK = 8
N_CORES = 8
TOK = B * S                  # 131072 tokens
TOKC = TOK // N_CORES        # 16384 tokens per core
GROUP = 128                  # tokens per compute group (SBUF partition dim)
BLK = 1024                   # tokens per DMA block
N_BLK = TOKC // BLK          # 16 blocks
GPB = BLK // GROUP           # 8 groups per block
DECAY_RATE = 0.99
LOG_CLAMP_MIN = -10.0

F32 = mybir.dt.float32


class _SplitDrainTileContext(tile.TileContext):
    """Walrus in this environment rejects >1-2 sem waits on the final SP
    Drain ("Too many sync wait commands").  Split the extra waits into
    standalone wait_ge instructions between the drain and the barrier —
    semantically identical (SP executes them in order before the barrier)."""

    _MAX_DRAIN_WAITS = 1

    def _drain_and_barrier(self, tick_clock, wait_clock):
        nc = self.nc
        drain_inst = nc.sync.drain()
        wait_clock.add_sem_waits(
            drain_inst.ins, ScopedClock({None: tick_clock.global_clock})
        )
        si = drain_inst.ins.sync_info
        waits = list(si.on_wait or [])
        if len(waits) > self._MAX_DRAIN_WAITS:
            si.on_wait = waits[: self._MAX_DRAIN_WAITS]
            assert self.sems is not None
            sems_by_name = {h.name: h for h in self.sems.allocated().values()}
            for w in waits[self._MAX_DRAIN_WAITS:]:
                h = sems_by_name.get(w.ant_name)
                assert h is not None, (w.ant_name, sorted(sems_by_name))
                assert w.wait_mode == "sem-ge-imm", w
                nc.sync.wait_ge(h, w.wait_value)
        nc.all_engine_barrier()
        assert self.sems is not None
        popped = nc._tile_sem_poison_stack.pop()
        assert popped is self._sem_poison
        nc.clear_and_free_semaphores(list(self.sems.allocated().values()))
        nc.all_engine_barrier()


_MAX_WAITS_PER_INST = 1
_wait_nop_counter = [0]


def _split_multi_waits(nc):
    """Walrus in this env rejects instructions carrying more than ~1 sem
    wait ("Too many sync wait commands").  Rewrite every instruction with
    k > _MAX_WAITS_PER_INST waits into (k-1) wait-only NoOps immediately
    before it on the same engine (engines execute their stream in order,
    so semantics are identical)."""
    for blk in nc.m.functions[0].blocks:
        insts = list(blk.instructions)
        out, changed = [], False
        for inst in insts:
            si = inst.sync_info
            waits = list(si.on_wait) if si and si.on_wait else []
            if len(waits) > _MAX_WAITS_PER_INST:
                changed = True
                for w in waits[:-_MAX_WAITS_PER_INST]:
                    _wait_nop_counter[0] += 1
                    out.append(mybir.InstNoOp(
                        name=f"WSPLIT-{_wait_nop_counter[0]}",
                        engine=inst.engine,
                        ins=[], outs=[],
                        sync_info=mybir.SyncInfo(on_wait=[w], on_update=[]),
                    ))
                si.on_wait = waits[-_MAX_WAITS_PER_INST:]
            out.append(inst)
        if changed:
            blk.instructions = out


def _build_program():
    """Build the per-core Bass program (identical on all 8 cores)."""
    nc = bass.Bass("TRN2", target_bir_lowering=False, debug=False)

    qT_d = nc.dram_tensor("qT", [D, TOKC], F32, kind="ExternalInput")
    p_d = nc.dram_tensor("pmat", [D, N], F32, kind="ExternalInput")
    bias_d = nc.dram_tensor("biasrow", [1, N], F32, kind="ExternalInput")
    mem_d = nc.dram_tensor("membank", [N, D], F32, kind="ExternalInput")
    ret_d = nc.dram_tensor("ret", [TOKC, D], F32, kind="ExternalOutput")
    attn_d = nc.dram_tensor("attn", [TOKC, N], F32, kind="ExternalOutput")

    # DRAM views for block-batched stores: token index = g*GROUP + p
    ret_view = ret_d.ap().rearrange("(b g p) d -> b p g d", p=GROUP, g=GPB)
    attn_view = attn_d.ap().rearrange("(b g p) n -> b p g n", p=GROUP, g=GPB)
    qT_view = qT_d.ap().rearrange("d (b t) -> d b t", t=BLK)

    with _SplitDrainTileContext(nc) as tc:
        with (
            tc.tile_pool(name="consts", bufs=1) as consts,
            tc.tile_pool(name="qin", bufs=2) as qin,
            tc.tile_pool(name="work", bufs=3) as work,
            tc.tile_pool(name="outstage", bufs=2) as outstage,
            tc.tile_pool(name="ps_s", bufs=3, space="PSUM") as ps_s_pool,
            tc.tile_pool(name="ps_at", bufs=2, space="PSUM") as ps_at_pool,
            tc.tile_pool(name="ps_r", bufs=2, space="PSUM") as ps_r_pool,
        ):
            # --- static setup -------------------------------------------------
            p0_sb = consts.tile([128, N], F32)
            p1_sb = consts.tile([128, N], F32)
            nc.sync.dma_start(p0_sb[:], p_d.ap()[0:128, :])
            nc.sync.dma_start(p1_sb[:], p_d.ap()[128:256, :])
            bias_sb = consts.tile([1, N], F32)
            nc.sync.dma_start(bias_sb[:], bias_d.ap())
            mem_sb = consts.tile([N, D], F32)
            nc.sync.dma_start(mem_sb[:], mem_d.ap())
            ones_sb = consts.tile([1, 128], F32)
            nc.vector.memset(ones_sb[:], 1.0)
            ident = consts.tile([128, 128], F32)
            make_identity(nc, ident[:])

            # --- main loop ----------------------------------------------------
            for b in range(N_BLK):
                qt0 = qin.tile([128, BLK], F32, tag="qt0")
                qt1 = qin.tile([128, BLK], F32, tag="qt1")
                nc.sync.dma_start(qt0[:], qT_view[0:128, b])
                nc.sync.dma_start(qt1[:], qT_view[128:256, b])

                attn_blk = outstage.tile([GROUP, GPB, N], F32, tag="attn_blk")
                ret_blk = outstage.tile([GROUP, GPB, D], F32, tag="ret_blk")

                for g in range(GPB):
                    tok = slice(g * GROUP, (g + 1) * GROUP)

                    # scores -> PSUM  (q @ P + bias)
                    ps_s = ps_s_pool.tile([GROUP, N], F32)
                    nc.tensor.matmul(ps_s[:], qt0[:, tok], p0_sb[:],
                                     start=True, stop=False)
                    nc.tensor.matmul(ps_s[:], qt1[:, tok], p1_sb[:],
                                     start=False, stop=False)
                    nc.tensor.matmul(ps_s[:], ones_sb[:], bias_sb[:],
                                     start=False, stop=True)

                    # em = exp(scores)   (ACT reads PSUM directly)
                    em = work.tile([GROUP, N], F32, tag="em")
                    nc.scalar.activation(em[:], ps_s[:],
                                         mybir.ActivationFunctionType.Exp)

                    # top-8 per row (exp is monotonic, so rank in em == rank in s)
                    e8 = work.tile([GROUP, 8], F32, tag="e8")
                    nc.vector.max(out=e8[:], in_=em[:])

                    # em_m = (em >= e8[:,7]) * em ; Z = rowsum(em_m)
                    em_m = work.tile([GROUP, N], F32, tag="em_m")
                    z = work.tile([GROUP, 1], F32, tag="z")
                    nc.vector.scalar_tensor_tensor(
                        out=em_m[:], in0=em[:], scalar=e8[:, 7:8], in1=em[:],
                        op0=mybir.AluOpType.is_ge, op1=mybir.AluOpType.mult,
                        accum_out=z[:],
                    )
                    r = work.tile([GROUP, 1], F32, tag="r")
                    nc.vector.reciprocal(r[:], z[:])

                    # attn = em_m * (1/Z)
                    attn_t = attn_blk[:, g, :]
                    nc.vector.tensor_scalar(
                        out=attn_t, in0=em_m[:], scalar1=r[:], scalar2=None,
                        op0=mybir.AluOpType.mult,
                    )

                    # attn^T via PE transpose -> [N, GROUP]
                    ps_at = ps_at_pool.tile([N, GROUP], F32)
                    nc.tensor.transpose(ps_at[:], attn_t, ident[:])
                    at_sb = work.tile([N, GROUP], F32, tag="at_sb")
                    nc.scalar.copy(at_sb[:], ps_at[:])

                    # retrieved = attn @ mem
                    ps_r = ps_r_pool.tile([GROUP, D], F32)
                    nc.tensor.matmul(ps_r[:], at_sb[:], mem_sb[:],
                                     start=True, stop=True)
                    nc.vector.tensor_copy(ret_blk[:, g, :], ps_r[:])

                # block-batched stores
                nc.sync.dma_start(attn_view[b], attn_blk[:])
                nc.sync.dma_start(ret_view[b], ret_blk[:])

    _split_multi_waits(nc)
    return nc


# ---------------------------------------------------------------------------
# Cached PJRT executable (mirrors concourse.bass2jax.run_bass_via_pjrt's
# multi-core branch, but jit-compiled once and reused across calls).
_CACHE: dict = {}


def _get_runner():
    if "runner" in _CACHE:
        return _CACHE["runner"]

    import jax
    from jax.sharding import Mesh, PartitionSpec
    from jax.experimental.shard_map import shard_map
    from concourse import bass2jax

    nc = _build_program()
    bass2jax.install_neuronx_cc_hook()

    partition_name = (
        nc.partition_id_tensor.name if nc.partition_id_tensor else None
    )
    in_names, out_names, out_avals, zero_shapes = [], [], [], []
    for alloc in nc.m.functions[0].allocations:
        if not isinstance(alloc, mybir.MemoryLocationSet):
            continue
        name = alloc.memorylocations[0].name
        if alloc.kind == "ExternalInput":
            if name != partition_name:
                in_names.append(name)
        elif alloc.kind == "ExternalOutput":
            out_names.append(name)
            shape = tuple(alloc.tensor_shape)
            dtype = mybir.dt.np(alloc.dtype)
            out_avals.append(jax.core.ShapedArray(shape, dtype))
            zero_shapes.append((shape, dtype))
    n_params = len(in_names)
    n_outs = len(out_names)
    all_in_names = in_names + out_names
    if partition_name is not None:
        all_in_names = all_in_names + [partition_name]

    def _body(*args):
        operands = list(args)
        if partition_name is not None:
            operands.append(bass2jax.partition_id_tensor())
        outs = bass2jax._bass_exec_p.bind(
            *operands,
            out_avals=tuple(out_avals),
            in_names=tuple(all_in_names),
            out_names=tuple(out_names),
            lowering_input_output_aliases=(),
            sim_require_finite=True,
            sim_require_nnan=True,
            nc=nc,
        )
        return tuple(outs)

    devices = jax.devices()[:N_CORES]
    assert len(devices) == N_CORES, f"need {N_CORES} cores, got {len(devices)}"
    mesh = Mesh(np.asarray(devices), ("core",))
    donate = tuple(range(n_params, n_params + n_outs))
    sharded = jax.jit(
        shard_map(
            _body, mesh=mesh,
            in_specs=(PartitionSpec("core"),) * (n_params + n_outs),
            out_specs=(PartitionSpec("core"),) * n_outs,
            check_rep=False,
        ),
        donate_argnums=donate,
        keep_unused=True,
    )

    def runner(per_core_inputs: list[dict]):
        concat_in = [
            np.concatenate([per_core_inputs[c][nm] for c in range(N_CORES)], axis=0)
            for nm in in_names
        ]
        concat_zeros = [
            np.zeros((N_CORES * sh[0], *sh[1:]), dt) for (sh, dt) in zero_shapes
        ]
        out_arrs = sharded(*concat_in, *concat_zeros)
        out_arrs = [np.asarray(a) for a in out_arrs]
        return [
            {
                nm: out_arrs[i].reshape(N_CORES, *out_avals[i].shape)[c]
                for i, nm in enumerate(out_names)
            }
            for c in range(N_CORES)
        ]

    runner.sharded = sharded
    runner.in_names = in_names
    runner.zero_shapes = zero_shapes
    _CACHE["runner"] = runner
    return runner


def _host_prep(query, memory, importance, age, W_q, W_k):
    """Host-side prep: tiny P/bias precompute + per-core q transpose."""
    mem0 = np.asarray(memory, dtype=np.float32)[0]            # [N, D]
    k = (np.asarray(mem0, dtype=np.float64)
         @ np.asarray(W_k, dtype=np.float64).T)               # [N, D]
    p_mat = (np.asarray(W_q, dtype=np.float64).T @ k.T) / np.sqrt(np.float64(D))
    p_mat = np.ascontiguousarray(p_mat, dtype=np.float32)     # [D, N]

    # bias exactly as the reference computes it (all f32 ops)
    imp = np.asarray(importance, dtype=np.float32)[0]
    agev = np.asarray(age, dtype=np.float32)[0]
    eff = imp * np.power(np.float32(DECAY_RATE), agev).astype(np.float32)
    bias = np.maximum(np.log(eff), np.float32(LOG_CLAMP_MIN)).astype(np.float32)
    bias = bias.reshape(1, N)

    qf = np.asarray(query, dtype=np.float32).reshape(TOK, D)
    per_core = []
    for c in range(N_CORES):
        q_slice = qf[c * TOKC:(c + 1) * TOKC]                 # [TOKC, D]
        qT = np.ascontiguousarray(q_slice.T)                  # [D, TOKC]
        per_core.append({
            "qT": qT,
            "pmat": p_mat,
            "biasrow": bias,
            "membank": np.ascontiguousarray(mem0),
        })
    return per_core


def kernel(query, memory, importance, age, W_q, W_k, top_k):
    assert int(top_k) == TOPK, f"kernel hardcodes top_k={TOPK}, got {top_k}"
    assert query.shape == (B, S, D), query.shape

    per_core = _host_prep(query, memory, importance, age, W_q, W_k)
    runner = _get_runner()
    results = runner(per_core)

    retrieved = np.concatenate(
        [results[c]["ret"] for c in range(N_CORES)], axis=0
    ).reshape(B, S, D)
    attn = np.concatenate(
        [results[c]["attn"] for c in range(N_CORES)], axis=0
    ).reshape(B, S, N)
    return retrieved.astype(np.float32), attn.astype(np.float32)


# revision 5
# speedup vs baseline: 61.6886x; 61.6886x over previous
"""Trainium2 Bass kernel for nn_MemoryBank (top-k masked attention over a
64-slot memory bank), data-parallel over 8 NeuronCores.

Math (per token t, memory slot n):
    scores[t, n] = (q[t] @ W_q.T) . (mem[n] @ W_k.T) / sqrt(D) + bias[n]
    bias[n]      = max(log(importance[n] * 0.99**age[n]), -10)
    top-8 mask over n, softmax, retrieved = attn @ mem

Device-side formulation:
    P = W_q.T @ (mem @ W_k.T).T / sqrt(D)    (256x64, host-precomputed in f64)
    scores = q @ P + bias                     (PE: 2 fp32 matmuls + K=1 bias matmul)
    em  = exp(scores)                         (ACT, direct from PSUM)
    e8  = top-8 of em per row                 (DVE max8; exp is monotonic)
    em_m = (em >= e8[:,7]) * em ; Z = sum     (one fused DVE scalar_tensor_tensor)
    attn = em_m * (1/Z)                       (DVE)
    retrieved = attn @ mem                    (PE transpose + matmul)

Sharding: batch-parallel. Core c gets tokens [c*16384, (c+1)*16384) of the
flattened (B*S=131072) token axis. q is host-transposed to [D, tokens] so the
contraction dim lands on SBUF partitions with dense DMA.
"""

import sys

for _p in ("/opt/trn_rl_repo",):
    if _p not in sys.path:
        sys.path.insert(0, _p)

import numpy as np

import concourse.bass as bass
import concourse.mybir as mybir
import concourse.tile as tile
from concourse.masks import make_identity
from concourse.vector_clock import ScopedClock

# ---------------------------------------------------------------------------
# Problem constants (hardcoded per the spec; kernel.py must be self-contained)
B, S, D, N = 32, 4096, 256, 64
TOPK = 8
N_CORES = 8
TOK = B * S                  # 131072 tokens
TOKC = TOK // N_CORES        # 16384 tokens per core
GROUP = 128                  # tokens per compute group (SBUF partition dim)
BLK = 1024                   # tokens per DMA block
N_BLK = TOKC // BLK          # 16 blocks
GPB = BLK // GROUP           # 8 groups per block
DECAY_RATE = 0.99
LOG_CLAMP_MIN = -10.0

F32 = mybir.dt.float32


class _SplitDrainTileContext(tile.TileContext):
    """Walrus in this environment rejects >1-2 sem waits on the final SP
    Drain ("Too many sync wait commands").  Split the extra waits into
    standalone wait_ge instructions between the drain and the barrier —
    semantically identical (SP executes them in order before the barrier)."""

    _MAX_DRAIN_WAITS = 1

    def _drain_and_barrier(self, tick_clock, wait_clock):
        nc = self.nc
        drain_inst = nc.sync.drain()
        wait_clock.add_sem_waits(
            drain_inst.ins, ScopedClock({None: tick_clock.global_clock})
        )
        si = drain_inst.ins.sync_info
        waits = list(si.on_wait or [])
        if len(waits) > self._MAX_DRAIN_WAITS:
            si.on_wait = waits[: self._MAX_DRAIN_WAITS]
            assert self.sems is not None
            sems_by_name = {h.name: h for h in self.sems.allocated().values()}
            for w in waits[self._MAX_DRAIN_WAITS:]:
                h = sems_by_name.get(w.ant_name)
                assert h is not None, (w.ant_name, sorted(sems_by_name))
                assert w.wait_mode == "sem-ge-imm", w
                nc.sync.wait_ge(h, w.wait_value)
        nc.all_engine_barrier()
        assert self.sems is not None
        popped = nc._tile_sem_poison_stack.pop()
        assert popped is self._sem_poison
        nc.clear_and_free_semaphores(list(self.sems.allocated().values()))
        nc.all_engine_barrier()


_MAX_WAITS_PER_INST = 1
_wait_nop_counter = [0]


def _split_multi_waits(nc):
    """Walrus in this env rejects instructions carrying more than ~1 sem
    wait ("Too many sync wait commands").  Rewrite every instruction with
    k > _MAX_WAITS_PER_INST waits into (k-1) wait-only NoOps immediately
    before it on the same engine (engines execute their stream in order,
    so semantics are identical)."""
    for blk in nc.m.functions[0].blocks:
        insts = list(blk.instructions)
        out, changed = [], False
        for inst in insts:
            si = inst.sync_info
            waits = list(si.on_wait) if si and si.on_wait else []
            if len(waits) > _MAX_WAITS_PER_INST:
                changed = True
                for w in waits[:-_MAX_WAITS_PER_INST]:
                    _wait_nop_counter[0] += 1
                    out.append(mybir.InstNoOp(
                        name=f"WSPLIT-{_wait_nop_counter[0]}",
                        engine=inst.engine,
                        ins=[], outs=[],
                        sync_info=mybir.SyncInfo(on_wait=[w], on_update=[]),
                    ))
                si.on_wait = waits[-_MAX_WAITS_PER_INST:]
            out.append(inst)
        if changed:
            blk.instructions = out


def _build_program():
    """Build the per-core Bass program (identical on all 8 cores)."""
    nc = bass.Bass("TRN2", target_bir_lowering=False, debug=False)

    qT_d = nc.dram_tensor("qT", [D, TOKC], F32, kind="ExternalInput")
    p_d = nc.dram_tensor("pmat", [D, N], F32, kind="ExternalInput")
    bias_d = nc.dram_tensor("biasrow", [1, N], F32, kind="ExternalInput")
    mem_d = nc.dram_tensor("membank", [N, D], F32, kind="ExternalInput")
    ret_d = nc.dram_tensor("ret", [TOKC, D], F32, kind="ExternalOutput")
    attn_d = nc.dram_tensor("attn", [TOKC, N], F32, kind="ExternalOutput")

    # DRAM views for block-batched stores: token index = g*GROUP + p
    ret_view = ret_d.ap().rearrange("(b g p) d -> b p g d", p=GROUP, g=GPB)
    attn_view = attn_d.ap().rearrange("(b g p) n -> b p g n", p=GROUP, g=GPB)
    qT_view = qT_d.ap().rearrange("d (b t) -> d b t", t=BLK)

    with _SplitDrainTileContext(nc) as tc:
        with (
            tc.tile_pool(name="consts", bufs=1) as consts,
            tc.tile_pool(name="qin", bufs=2) as qin,
            tc.tile_pool(name="work", bufs=3) as work,
            tc.tile_pool(name="outstage", bufs=2) as outstage,
            tc.tile_pool(name="ps_s", bufs=3, space="PSUM") as ps_s_pool,
            tc.tile_pool(name="ps_at", bufs=2, space="PSUM") as ps_at_pool,
            tc.tile_pool(name="ps_r", bufs=2, space="PSUM") as ps_r_pool,
        ):
            # --- static setup -------------------------------------------------
            p0_sb = consts.tile([128, N], F32)
            p1_sb = consts.tile([128, N], F32)
            nc.sync.dma_start(p0_sb[:], p_d.ap()[0:128, :])
            nc.sync.dma_start(p1_sb[:], p_d.ap()[128:256, :])
            bias_sb = consts.tile([1, N], F32)
            nc.sync.dma_start(bias_sb[:], bias_d.ap())
            mem_sb = consts.tile([N, D], F32)
            nc.sync.dma_start(mem_sb[:], mem_d.ap())
            ones_sb = consts.tile([1, 128], F32)
            nc.vector.memset(ones_sb[:], 1.0)
            ident = consts.tile([128, 128], F32)
            make_identity(nc, ident[:])

            # --- main loop ----------------------------------------------------
            for b in range(N_BLK):
                qt0 = qin.tile([128, BLK], F32, tag="qt0")
                qt1 = qin.tile([128, BLK], F32, tag="qt1")
                nc.sync.dma_start(qt0[:], qT_view[0:128, b])
                nc.sync.dma_start(qt1[:], qT_view[128:256, b])

                attn_blk = outstage.tile([GROUP, GPB, N], F32, tag="attn_blk")
                ret_blk = outstage.tile([GROUP, GPB, D], F32, tag="ret_blk")

                for g in range(GPB):
                    tok = slice(g * GROUP, (g + 1) * GROUP)

                    # scores -> PSUM  (q @ P + bias)
                    ps_s = ps_s_pool.tile([GROUP, N], F32)
                    nc.tensor.matmul(ps_s[:], qt0[:, tok], p0_sb[:],
                                     start=True, stop=False)
                    nc.tensor.matmul(ps_s[:], qt1[:, tok], p1_sb[:],
                                     start=False, stop=False)
                    nc.tensor.matmul(ps_s[:], ones_sb[:], bias_sb[:],
                                     start=False, stop=True)

                    # em = exp(scores)   (ACT reads PSUM directly)
                    em = work.tile([GROUP, N], F32, tag="em")
                    nc.scalar.activation(em[:], ps_s[:],
                                         mybir.ActivationFunctionType.Exp)

                    # top-8 per row (exp is monotonic, so rank in em == rank in s)
                    e8 = work.tile([GROUP, 8], F32, tag="e8")
                    nc.vector.max(out=e8[:], in_=em[:])

                    # em_m = (em >= e8[:,7]) * em ; Z = rowsum(em_m)
                    em_m = work.tile([GROUP, N], F32, tag="em_m")
                    z = work.tile([GROUP, 1], F32, tag="z")
                    nc.vector.scalar_tensor_tensor(
                        out=em_m[:], in0=em[:], scalar=e8[:, 7:8], in1=em[:],
                        op0=mybir.AluOpType.is_ge, op1=mybir.AluOpType.mult,
                        accum_out=z[:],
                    )
                    r = work.tile([GROUP, 1], F32, tag="r")
                    nc.vector.reciprocal(r[:], z[:])

                    # attn = em_m * (1/Z)
                    attn_t = attn_blk[:, g, :]
                    nc.vector.tensor_scalar(
                        out=attn_t, in0=em_m[:], scalar1=r[:], scalar2=None,
                        op0=mybir.AluOpType.mult,
                    )

                    # attn^T via PE transpose -> [N, GROUP]
                    ps_at = ps_at_pool.tile([N, GROUP], F32)
                    nc.tensor.transpose(ps_at[:], attn_t, ident[:])
                    at_sb = work.tile([N, GROUP], F32, tag="at_sb")
                    nc.scalar.copy(at_sb[:], ps_at[:])

                    # retrieved = attn @ mem
                    ps_r = ps_r_pool.tile([GROUP, D], F32)
                    nc.tensor.matmul(ps_r[:], at_sb[:], mem_sb[:],
                                     start=True, stop=True)
                    nc.vector.tensor_copy(ret_blk[:, g, :], ps_r[:])

                # block-batched stores
                nc.sync.dma_start(attn_view[b], attn_blk[:])
                nc.sync.dma_start(ret_view[b], ret_blk[:])

    _split_multi_waits(nc)
    return nc


# ---------------------------------------------------------------------------
# Cached PJRT executable (mirrors concourse.bass2jax.run_bass_via_pjrt's
# multi-core branch, but jit-compiled once and reused across calls).
_CACHE: dict = {}


def _get_runner():
    if "runner" in _CACHE:
        return _CACHE["runner"]

    import jax
    from jax.sharding import Mesh, PartitionSpec
    from jax.experimental.shard_map import shard_map
    from concourse import bass2jax

    nc = _build_program()
    bass2jax.install_neuronx_cc_hook()

    partition_name = (
        nc.partition_id_tensor.name if nc.partition_id_tensor else None
    )
    in_names, out_names, out_avals, zero_shapes = [], [], [], []
    for alloc in nc.m.functions[0].allocations:
        if not isinstance(alloc, mybir.MemoryLocationSet):
            continue
        name = alloc.memorylocations[0].name
        if alloc.kind == "ExternalInput":
            if name != partition_name:
                in_names.append(name)
        elif alloc.kind == "ExternalOutput":
            out_names.append(name)
            shape = tuple(alloc.tensor_shape)
            dtype = mybir.dt.np(alloc.dtype)
            out_avals.append(jax.core.ShapedArray(shape, dtype))
            zero_shapes.append((shape, dtype))
    n_params = len(in_names)
    n_outs = len(out_names)
    all_in_names = in_names + out_names
    if partition_name is not None:
        all_in_names = all_in_names + [partition_name]

    def _body(*args):
        operands = list(args)
        if partition_name is not None:
            operands.append(bass2jax.partition_id_tensor())
        outs = bass2jax._bass_exec_p.bind(
            *operands,
            out_avals=tuple(out_avals),
            in_names=tuple(all_in_names),
            out_names=tuple(out_names),
            lowering_input_output_aliases=(),
            sim_require_finite=True,
            sim_require_nnan=True,
            nc=nc,
        )
        return tuple(outs)

    devices = jax.devices()[:N_CORES]
    assert len(devices) == N_CORES, f"need {N_CORES} cores, got {len(devices)}"
    mesh = Mesh(np.asarray(devices), ("core",))
    donate = tuple(range(n_params, n_params + n_outs))
    sharded = jax.jit(
        shard_map(
            _body, mesh=mesh,
            in_specs=(PartitionSpec("core"),) * (n_params + n_outs),
            out_specs=(PartitionSpec("core"),) * n_outs,
            check_rep=False,
        ),
        donate_argnums=donate,
        keep_unused=True,
    )

    def runner(per_core_inputs: list[dict]):
        concat_in = [
            np.concatenate([per_core_inputs[c][nm] for c in range(N_CORES)], axis=0)
            for nm in in_names
        ]
        concat_zeros = [
            np.zeros((N_CORES * sh[0], *sh[1:]), dt) for (sh, dt) in zero_shapes
        ]
        out_arrs = sharded(*concat_in, *concat_zeros)
        out_arrs = [np.asarray(a) for a in out_arrs]
        return [
            {
                nm: out_arrs[i].reshape(N_CORES, *out_avals[i].shape)[c]
                for i, nm in enumerate(out_names)
            }
            for c in range(N_CORES)
        ]

    runner.sharded = sharded
    runner.in_names = in_names
    runner.zero_shapes = zero_shapes
    runner.body = _body
    runner.mesh = mesh
    runner.n_params = n_params
    runner.n_outs = n_outs
    _CACHE["runner"] = runner
    return runner


def _host_prep(query, memory, importance, age, W_q, W_k):
    """Host-side prep: tiny P/bias precompute + per-core q transpose."""
    mem0 = np.asarray(memory, dtype=np.float32)[0]            # [N, D]
    k = (np.asarray(mem0, dtype=np.float64)
         @ np.asarray(W_k, dtype=np.float64).T)               # [N, D]
    p_mat = (np.asarray(W_q, dtype=np.float64).T @ k.T) / np.sqrt(np.float64(D))
    p_mat = np.ascontiguousarray(p_mat, dtype=np.float32)     # [D, N]

    # bias exactly as the reference computes it (all f32 ops)
    imp = np.asarray(importance, dtype=np.float32)[0]
    agev = np.asarray(age, dtype=np.float32)[0]
    eff = imp * np.power(np.float32(DECAY_RATE), agev).astype(np.float32)
    bias = np.maximum(np.log(eff), np.float32(LOG_CLAMP_MIN)).astype(np.float32)
    bias = bias.reshape(1, N)

    qf = np.asarray(query, dtype=np.float32).reshape(TOK, D)
    per_core = []
    for c in range(N_CORES):
        q_slice = qf[c * TOKC:(c + 1) * TOKC]                 # [TOKC, D]
        qT = np.ascontiguousarray(q_slice.T)                  # [D, TOKC]
        per_core.append({
            "qT": qT,
            "pmat": p_mat,
            "biasrow": bias,
            "membank": np.ascontiguousarray(mem0),
        })
    return per_core


def kernel(query, memory, importance, age, W_q, W_k, top_k):
    assert int(top_k) == TOPK, f"kernel hardcodes top_k={TOPK}, got {top_k}"
    assert query.shape == (B, S, D), query.shape

    per_core = _host_prep(query, memory, importance, age, W_q, W_k)
    runner = _get_runner()
    results = runner(per_core)

    retrieved = np.concatenate(
        [results[c]["ret"] for c in range(N_CORES)], axis=0
    ).reshape(B, S, D)
    attn = np.concatenate(
        [results[c]["attn"] for c in range(N_CORES)], axis=0
    ).reshape(B, S, N)
    return retrieved.astype(np.float32), attn.astype(np.float32)
